# revision 25
# baseline (speedup 1.0000x reference)
"""Trainium2 Bass kernel for EnergyIrrepModulation.

Computes out[m, e, d] = x[m, d] * gates_full[e, d] where
gates = MLP(e_feat) : [nE, n_copies], expanded to [nE, D] via the static
irrep index map for IRREPS = [(64, 1), (32, 3), (16, 5)].

Sharding: data-parallel over M (4096 rows -> 512 rows per core, 8 cores).
Gates/MLP params are replicated; each core redundantly computes the tiny MLP.

The kernel is HBM-bound: the only real cost is materializing the
[M, nE, D] output. The main data path runs in bf16 (x, expanded gates,
output) to halve that traffic; the harness tolerance (2e-2) dwarfs bf16
rounding (~7e-3 measured). MLP weights are bf16 (PE accumulates fp32);
biases and PSUM stay fp32.

Per-core device plan:
  1. MLP weights packed in ONE [128, 1076] bf16 tensor (two DMAs, first
     layer's weights land first) + a tiny [128, 4] f32 bias tensor; host
     pre-transposes e_feat.
  2. Tiny MLP on the tensor engine; biases+ReLU fused on the scalar
     engine (b3 added along the free dim with a ones[1,100]^T @ b3[1,112]
     matmul).
  3. Gates [100, 112] (PSUM) -> expanded [100, 240] bf16 via three ACT
     copies that fuse the irrep 112->240 broadcast into the access
     pattern.
  4. Expanded gates broadcast to all 128 partitions in 6 e-chunks of
     1/4/10/20/30/35 e's, a ladder sized so each chunk lands just before
     its multiplies need it:
       - chunk 0 (e=0): gexp[0:1,:] is already a partition-0 row, so a
         single GPSIMD partition_broadcast with no flatten -> first store
         by ~16 us.
       - chunk 1 (e 1:5): flatten to partition 0 + GPSIMD
         partition_broadcast, done before the loop heats up (GPSIMD
         shares an SBUF port with the DVE; overlap slows both ~3x).
       - chunks 2..5: PE+ACT — flatten [95,240] onto partition 0, then
         per 512-col piece matmul ones[1,128]^T @ flat[1,512] into PSUM
         and ACT-copy PSUM -> SBUF bf16. No DMA bytes, no DVE contention
         (a DRAM-bounce broadcast measured ~2x its byte count in DMA
         time).
  5. Main loop (chunk-major): the vector engine multiplies x [128, 240]
     bf16 (stride-0 read over the e axis) against the expanded gates.
     All operands bf16 with unit innermost stride -> DVE 2x_1P mode
     (~53 us busy, hidden under the stores).
  6. Stores: one DMA per (chunk, m-tile), alternating between both HWDGE
     rings. ~24.6 MB per core at the ~394 GB/s per-core rate measured.
"""

import sys
from contextlib import ExitStack

import numpy as np

try:
    import concourse.bass as bass  # noqa: F401
except ImportError:  # pragma: no cover
    sys.path.insert(0, "/opt/trn_rl_repo")
    import concourse.bass as bass

import ml_dtypes

import concourse.bacc as bacc
import concourse.tile as tile
from concourse import mybir
from concourse.bass_utils import run_bass_kernel_spmd

FP32 = mybir.dt.float32
BF16 = mybir.dt.bfloat16
BF16_NP = ml_dtypes.bfloat16

M, D = 4096, 240
NE, E_DIM, HIDDEN, NCOP = 100, 64, 256, 112
N_CORES = 8
MC = M // N_CORES          # 512 rows per core
MT = MC // 128             # 4 m-tiles of 128 rows
# e-chunk ladder: tiny chunks first so stores start ASAP, later chunks
# grow as the PE+ACT broadcast pipeline runs ahead of the multiplies
CHUNKS = [(0, 1), (1, 5), (5, 15), (15, 35), (35, 65), (65, 100)]
NGP = 2                    # chunks 0..NGP-1 broadcast via GPSIMD

# packed bf16 weight layout (columns of the [128, NPW] tensor)
PW_W2A, PW_W2B = 0, 256
PW_W3A, PW_W3B = 512, 624
PW_W1 = 736                # [64, 128] x 2 stacked on partition halves
PW_ET = 864                # e_featT [64, 100] duplicated on both halves
PW_B3 = 964                # [1, 112] on partition 0
NPW = 1076
NPF = 4                    # f32 biases: b1 (2 cols), b2 (2 cols)

_CACHE = {}


def _build_program():
    nc = bacc.Bacc(None, target_bir_lowering=False, debug=False)

    x_d = nc.dram_tensor("x", [MC, D], BF16, kind="ExternalInput")
    pw_d = nc.dram_tensor("pweights", [128, NPW], BF16, kind="ExternalInput")
    pf_d = nc.dram_tensor("pbias", [128, NPF], FP32, kind="ExternalInput")
    out_d = nc.dram_tensor("out", [MC, NE * D], BF16, kind="ExternalOutput")

    with tile.TileContext(nc) as tc, ExitStack() as ctx:
        const_pool = ctx.enter_context(tc.tile_pool(name="const", bufs=1))
        mlp_pool = ctx.enter_context(tc.tile_pool(name="mlp", bufs=1))
        mlp_psum_ctx = ExitStack()
        psum_mlp = mlp_psum_ctx.enter_context(
            tc.tile_pool(name="psum_mlp", bufs=2, space="PSUM")
        )
        st_pool = ctx.enter_context(tc.tile_pool(name="stage", bufs=1))
        raw_pool = ctx.enter_context(tc.tile_pool(name="raw", bufs=1))
        x_pool = ctx.enter_context(tc.tile_pool(name="xin", bufs=1))
        out_pool = ctx.enter_context(tc.tile_pool(name="out", bufs=5))

        pw_t = const_pool.tile([128, NPW], BF16)
        pf_t = const_pool.tile([128, NPF], FP32)
        # critical first-layer params (W1, eT, b3) land first
        nc.sync.dma_start(out=pw_t[:, PW_W1:NPW], in_=pw_d[:, PW_W1:NPW])
        nc.sync.dma_start(out=pf_t[:], in_=pf_d[:])
        nc.scalar.dma_start(out=pw_t[:, 0:PW_W1], in_=pw_d[:, 0:PW_W1])
        ones_bf = const_pool.tile([1, 128], BF16)
        nc.vector.memset(ones_bf[:], 1.0)

        # all four x m-tiles loaded upfront (0.25 MB total, long before the
        # stores saturate the rings)
        x_t = []
        for mt in range(MT):
            xt = x_pool.tile([128, D], BF16, tag=f"x{mt}", name=f"x{mt}")
            eng = nc.sync if mt % 2 == 0 else nc.scalar
            eng.dma_start(out=xt[:], in_=x_d[mt * 128 : (mt + 1) * 128, :])
            x_t.append(xt)

        relu = mybir.ActivationFunctionType.Relu

        # ---- MLP: h1T = relu(W1^T e_featT + b1), two [128, 100] tiles ----
        h1T = []
        for c in range(2):
            pl, ph = 64 * c, 64 * (c + 1)
            ps = psum_mlp.tile([128, NE], FP32)
            nc.tensor.matmul(
                ps[:], pw_t[pl:ph, PW_W1 : PW_W1 + 128], pw_t[pl:ph, PW_ET : PW_ET + NE],
                start=True, stop=True,
            )
            h = mlp_pool.tile([128, NE], BF16, tag=f"h1T{c}")
            nc.scalar.activation(h[:], ps[:], relu, bias=pf_t[:, c : c + 1])
            h1T.append(h)

        # ---- h2T = relu(W2^T h1T + b2) ----
        h2T = []
        for c in range(2):
            ps = psum_mlp.tile([128, NE], FP32)
            nc.tensor.matmul(
                ps[:], pw_t[:, PW_W2A + c * 128 : PW_W2A + (c + 1) * 128], h1T[0][:],
                start=True, stop=False,
            )
            nc.tensor.matmul(
                ps[:], pw_t[:, PW_W2B + c * 128 : PW_W2B + (c + 1) * 128], h1T[1][:],
                start=False, stop=True,
            )
            h = mlp_pool.tile([128, NE], BF16, tag=f"h2T{c}")
            nc.scalar.activation(h[:], ps[:], relu, bias=pf_t[:, 2 + c : 3 + c])
            h2T.append(h)

        # ---- gates = h2 @ W3 + b3 : psum [100, 112], partition = e ----
        psg = psum_mlp.tile([NE, NCOP], FP32)
        nc.tensor.matmul(
            psg[:], h2T[0][:], pw_t[:, PW_W3A : PW_W3A + NCOP], start=True, stop=False
        )
        nc.tensor.matmul(
            psg[:], h2T[1][:], pw_t[:, PW_W3B : PW_W3B + NCOP], start=False, stop=False
        )
        # += ones[100,1] @ b3[1,112]: bias along the free dim via PE
        nc.tensor.matmul(
            psg[:], ones_bf[0:1, 0:NE], pw_t[0:1, PW_B3 : PW_B3 + NCOP],
            start=False, stop=True,
        )

        # ---- expand gates [100, 112] -> [100, 240] bf16 (irrep index map)
        # fused into PSUM->SBUF copies with broadcast source APs. Row 0 is
        # expanded first so chunk 0's partition_broadcast (and with it the
        # first store) doesn't wait on the full expansion.
        # (engines can't start a pattern at a non-zero partition for >32
        # rows, so the second pass redundantly re-covers rows 0:32)
        gexp = mlp_pool.tile([NE, D], BF16, tag="gexp")
        for r0, r1 in [(0, 32), (0, NE)]:
            nr = r1 - r0
            nc.scalar.copy(gexp[r0:r1, 0:64], psg[r0:r1, 0:64])
            nc.scalar.copy(
                gexp[r0:r1, 64:160].rearrange("e (i k) -> e i k", k=3),
                psg[r0:r1, 64:96].unsqueeze(2).to_broadcast((nr, 32, 3)),
            )
            nc.scalar.copy(
                gexp[r0:r1, 160:240].rearrange("e (i k) -> e i k", k=5),
                psg[r0:r1, 96:112].unsqueeze(2).to_broadcast((nr, 16, 5)),
            )

        # MLP PSUM banks are no longer needed; free them so the broadcast
        # pool below can use 2x4 banks (2048-col pieces).
        mlp_psum_ctx.close()
        psum_bc = ctx.enter_context(
            tc.tile_pool(name="psum_bc", bufs=2, space="PSUM")
        )

        # ---- broadcast expanded gates to all 128 partitions (see module
        # docstring): chunk 0 straight off partition 0, chunk 1 via
        # flatten + GPSIMD, chunks 2+ via PE+ACT pieces.
        raws = []
        for ci, (lo, hi) in enumerate(CHUNKS):
            raw = raw_pool.tile([128, (hi - lo) * D], BF16, tag=f"raw{ci}")
            raws.append(raw)

        nc.gpsimd.partition_broadcast(raws[0][:], gexp[0:1, :])

        lo1, hi1 = CHUNKS[1]
        st = st_pool.tile([1, (hi1 - lo1) * D], BF16)
        nc.scalar.dma_start(out=st[:], in_=gexp[lo1:hi1, :])
        nc.gpsimd.partition_broadcast(raws[1][:], st[0:1, :])

        fbase = CHUNKS[NGP][0]  # first PE+ACT e-row
        flat = st_pool.tile([1, (NE - fbase) * D], BF16)
        nc.sync.dma_start(out=flat[:], in_=gexp[fbase:NE, :])

        def bcast_pieces(ci):
            """PE+ACT broadcast of chunk ci (2048-col PSUM pieces)."""
            lo, hi = CHUNKS[ci]
            n = (hi - lo) * D
            base = (lo - fbase) * D
            for p0 in range(0, n, 2048):
                w = min(2048, n - p0)
                ps = psum_bc.tile(
                    [128, 2048], FP32, name=f"psbc{ci}_{p0}", tag="psbc"
                )
                for q0 in range(0, w, 512):
                    qw = min(512, w - q0)
                    nc.tensor.matmul(
                        ps[:, q0 : q0 + qw],
                        ones_bf[0:1, :],
                        flat[0:1, base + p0 + q0 : base + p0 + q0 + qw],
                        start=True, stop=True,
                    )
                nc.scalar.copy(raws[ci][:, p0 : p0 + w], ps[:, 0:w])

        # Emit ALL piece copies upfront, back-to-back: the ACT ring carries
        # no store DMAs (those all go on the sync ring), so the copies
        # stream 17..46 us and each chunk lands just before its multiplies
        # need it. (Interleaving copies between store DMA instructions
        # delayed chunk E's broadcast ~7 us and starved the store stream.)
        for ci in range(NGP, len(CHUNKS)):
            bcast_pieces(ci)

        # ---- main loop (chunk-major): out[m, e, d] = x[m, d] * gexp[e, d]
        # Chunk-major order so each chunk's multiplies start right after its
        # broadcast lands; stores alternate between both HWDGE rings. The
        # broadcast for chunk ci+2 is emitted between chunk ci's stores so
        # ACT-ring stores don't queue behind a full chunk of copies.
        si = 0
        for ci, (lo, hi) in enumerate(CHUNKS):
            ec = hi - lo
            n = ec * D
            for mt in range(MT):
                o_t = out_pool.tile([128, n], BF16)
                nc.vector.tensor_mul(
                    o_t[:].rearrange("p (e d) -> p e d", d=D),
                    x_t[mt][:].unsqueeze(1).to_broadcast((128, ec, D)),
                    raws[ci][:].rearrange("p (e d) -> p e d", d=D),
                )
                nc.sync.dma_start(
                    out=out_d[mt * 128 : (mt + 1) * 128, lo * D : hi * D],
                    in_=o_t[:],
                )
                si += 1

    nc.compile()
    return nc


def _marshal(inputs):
    f32 = lambda a: np.ascontiguousarray(np.asarray(a, dtype=np.float32))
    bf = lambda a: f32(a).astype(BF16_NP)
    x = bf(inputs["x"])
    W1, W2, W3 = bf(inputs["W1"]), bf(inputs["W2"]), bf(inputs["W3"])
    b1, b2, b3 = f32(inputs["b1"]), f32(inputs["b2"]), f32(inputs["b3"])
    eT = bf(np.asarray(inputs["e_feat"]).T)

    pw = np.zeros((128, NPW), BF16_NP)
    pw[:, PW_W2A : PW_W2A + 256] = W2[0:128]
    pw[:, PW_W2B : PW_W2B + 256] = W2[128:256]
    pw[:, PW_W3A : PW_W3A + NCOP] = W3[0:128]
    pw[:, PW_W3B : PW_W3B + NCOP] = W3[128:256]
    pw[0:64, PW_W1 : PW_W1 + 128] = W1[:, 0:128]
    pw[64:128, PW_W1 : PW_W1 + 128] = W1[:, 128:256]
    pw[0:64, PW_ET : PW_ET + NE] = eT
    pw[64:128, PW_ET : PW_ET + NE] = eT
    pw[0, PW_B3 : PW_B3 + NCOP] = b3.astype(BF16_NP)

    pf = np.zeros((128, NPF), np.float32)
    pf[:, 0] = b1[0:128]
    pf[:, 1] = b1[128:256]
    pf[:, 2] = b2[0:128]
    pf[:, 3] = b2[128:256]

    return [
        {"x": x[i * MC : (i + 1) * MC], "pweights": pw, "pbias": pf}
        for i in range(N_CORES)
    ]


def get_program():
    if "nc" not in _CACHE:
        _CACHE["nc"] = _build_program()
    return _CACHE["nc"]


def run(inputs, trace=False, **kwargs):
    """Run on 8 cores; returns (out [M, NE, D] f32, BassKernelResults)."""
    nc = get_program()
    in_maps = _marshal(inputs)
    res = run_bass_kernel_spmd(
        nc, in_maps, core_ids=list(range(N_CORES)), trace=trace, **kwargs
    )
    out = np.concatenate(
        [
            np.asarray(res.results[i]["out"])
            .astype(np.float32)
            .reshape(MC, NE, D)
            for i in range(N_CORES)
        ],
        axis=0,
    )
    return out, res


def kernel(**inputs) -> np.ndarray:
    out, _ = run(inputs)
    return out


# revision 31
# speedup vs baseline: 1.1595x; 1.1595x over previous
"""Trainium2 Bass kernel for EnergyIrrepModulation.

Computes out[m, e, d] = x[m, d] * gates_full[e, d] where
gates = MLP(e_feat) : [nE, n_copies], expanded to [nE, D] via the static
irrep index map for IRREPS = [(64, 1), (32, 3), (16, 5)].

Sharding: data-parallel over M (4096 rows -> 512 rows per core, 8 cores).
Gates/MLP params are replicated; each core redundantly computes the tiny MLP.

The kernel is HBM-bound: the only real cost is materializing the
[M, nE, D] output. The main data path runs in bf16 (x, expanded gates,
output) to halve that traffic; the harness tolerance (2e-2) dwarfs bf16
rounding (~7e-3 measured). MLP weights are bf16 (PE accumulates fp32);
biases and PSUM stay fp32.

Per-core device plan:
  1. MLP weights packed in ONE [128, 1076] bf16 tensor (two DMAs, first
     layer's weights land first) + a tiny [128, 4] f32 bias tensor; host
     pre-transposes e_feat.
  2. Tiny MLP on the tensor engine; biases+ReLU fused on the scalar
     engine (b3 added along the free dim with a ones[1,100]^T @ b3[1,112]
     matmul).
  3. Gates [100, 112] (PSUM) -> expanded [100, 240] bf16 via three ACT
     copies that fuse the irrep 112->240 broadcast into the access
     pattern.
  4. Expanded gates broadcast to all 128 partitions in 6 e-chunks of
     1/4/10/20/30/35 e's, a ladder sized so each chunk lands just before
     its multiplies need it:
       - chunk 0 (e=0): gexp[0:1,:] is already a partition-0 row, so a
         single GPSIMD partition_broadcast with no flatten -> first store
         by ~16 us.
       - chunk 1 (e 1:5): flatten to partition 0 + GPSIMD
         partition_broadcast, done before the loop heats up (GPSIMD
         shares an SBUF port with the DVE; overlap slows both ~3x).
       - chunks 2..5: PE+ACT — flatten [95,240] onto partition 0, then
         per 512-col piece matmul ones[1,128]^T @ flat[1,512] into PSUM
         and ACT-copy PSUM -> SBUF bf16. No DMA bytes, no DVE contention
         (a DRAM-bounce broadcast measured ~2x its byte count in DMA
         time).
  5. Main loop (chunk-major): the vector engine multiplies x [128, 240]
     bf16 (stride-0 read over the e axis) against the expanded gates.
     All operands bf16 with unit innermost stride -> DVE 2x_1P mode
     (~53 us busy, hidden under the stores).
  6. Stores: one DMA per (chunk, m-tile), alternating between both HWDGE
     rings. ~24.6 MB per core at the ~394 GB/s per-core rate measured.
"""

import sys
from contextlib import ExitStack

import numpy as np

try:
    import concourse.bass as bass  # noqa: F401
except ImportError:  # pragma: no cover
    sys.path.insert(0, "/opt/trn_rl_repo")
    import concourse.bass as bass

import ml_dtypes

import concourse.bacc as bacc
import concourse.tile as tile
from concourse import mybir
from concourse.bass_utils import run_bass_kernel_spmd

FP32 = mybir.dt.float32
BF16 = mybir.dt.bfloat16
BF16_NP = ml_dtypes.bfloat16

M, D = 4096, 240
NE, E_DIM, HIDDEN, NCOP = 100, 64, 256, 112
N_CORES = 8
MC = M // N_CORES          # 512 rows per core
MT = MC // 128             # 4 m-tiles of 128 rows
# e-chunk ladder: tiny chunks first so stores start ASAP, later chunks
# grow as the PE+ACT broadcast pipeline runs ahead of the multiplies
CHUNKS = [(0, 1), (1, 5), (5, 15), (15, 35), (35, 65), (65, 100)]
NGP = 3                    # chunks 0..NGP-1 broadcast via GPSIMD

# packed bf16 weight layout (columns of the [128, NPW] tensor)
PW_W2A, PW_W2B = 0, 256
PW_W3A, PW_W3B = 512, 624
PW_W1 = 736                # [64, 128] x 2 stacked on partition halves
PW_ET = 864                # e_featT [64, 100] duplicated on both halves
PW_B3 = 964                # [1, 112] on partition 0
NPW = 1076
NPF = 4                    # f32 biases: b1 (2 cols), b2 (2 cols)

_CACHE = {}


def _build_program():
    nc = bacc.Bacc(None, target_bir_lowering=False, debug=False)

    x_d = nc.dram_tensor("x", [MC, D], BF16, kind="ExternalInput")
    pw_d = nc.dram_tensor("pweights", [128, NPW], BF16, kind="ExternalInput")
    pf_d = nc.dram_tensor("pbias", [128, NPF], FP32, kind="ExternalInput")
    out_d = nc.dram_tensor("out", [MC, NE * D], BF16, kind="ExternalOutput")

    with tile.TileContext(nc) as tc, ExitStack() as ctx:
        const_pool = ctx.enter_context(tc.tile_pool(name="const", bufs=1))
        mlp_pool = ctx.enter_context(tc.tile_pool(name="mlp", bufs=1))
        psum_mlp = ctx.enter_context(
            tc.tile_pool(name="psum_mlp", bufs=2, space="PSUM")
        )
        psum_bc = ctx.enter_context(
            tc.tile_pool(name="psum_bc", bufs=2, space="PSUM")
        )
        st_pool = ctx.enter_context(tc.tile_pool(name="stage", bufs=1))
        raw_pool = ctx.enter_context(tc.tile_pool(name="raw", bufs=1))
        x_pool = ctx.enter_context(tc.tile_pool(name="xin", bufs=1))
        out_pool = ctx.enter_context(tc.tile_pool(name="out", bufs=5))

        pw_t = const_pool.tile([128, NPW], BF16)
        pf_t = const_pool.tile([128, NPF], FP32)
        # critical first-layer params (W1, eT, b3) land first
        nc.sync.dma_start(out=pw_t[:, PW_W1:NPW], in_=pw_d[:, PW_W1:NPW])
        nc.sync.dma_start(out=pf_t[:], in_=pf_d[:])
        nc.scalar.dma_start(out=pw_t[:, 0:PW_W1], in_=pw_d[:, 0:PW_W1])
        ones_bf = const_pool.tile([1, 128], BF16)
        nc.vector.memset(ones_bf[:], 1.0)

        # all four x m-tiles loaded upfront (0.25 MB total, long before the
        # stores saturate the rings)
        x_t = []
        for mt in range(MT):
            xt = x_pool.tile([128, D], BF16, tag=f"x{mt}", name=f"x{mt}")
            eng = nc.sync if mt % 2 == 0 else nc.scalar
            eng.dma_start(out=xt[:], in_=x_d[mt * 128 : (mt + 1) * 128, :])
            x_t.append(xt)

        relu = mybir.ActivationFunctionType.Relu

        # ---- MLP: h1T = relu(W1^T e_featT + b1), two [128, 100] tiles ----
        h1T = []
        for c in range(2):
            pl, ph = 64 * c, 64 * (c + 1)
            ps = psum_mlp.tile([128, NE], FP32)
            nc.tensor.matmul(
                ps[:], pw_t[pl:ph, PW_W1 : PW_W1 + 128], pw_t[pl:ph, PW_ET : PW_ET + NE],
                start=True, stop=True,
            )
            h = mlp_pool.tile([128, NE], BF16, tag=f"h1T{c}")
            nc.scalar.activation(h[:], ps[:], relu, bias=pf_t[:, c : c + 1])
            h1T.append(h)

        # ---- h2T = relu(W2^T h1T + b2) ----
        h2T = []
        for c in range(2):
            ps = psum_mlp.tile([128, NE], FP32)
            nc.tensor.matmul(
                ps[:], pw_t[:, PW_W2A + c * 128 : PW_W2A + (c + 1) * 128], h1T[0][:],
                start=True, stop=False,
            )
            nc.tensor.matmul(
                ps[:], pw_t[:, PW_W2B + c * 128 : PW_W2B + (c + 1) * 128], h1T[1][:],
                start=False, stop=True,
            )
            h = mlp_pool.tile([128, NE], BF16, tag=f"h2T{c}")
            nc.scalar.activation(h[:], ps[:], relu, bias=pf_t[:, 2 + c : 3 + c])
            h2T.append(h)

        # ---- gates = h2 @ W3 + b3 : psum [100, 112], partition = e ----
        psg = psum_mlp.tile([NE, NCOP], FP32)
        nc.tensor.matmul(
            psg[:], h2T[0][:], pw_t[:, PW_W3A : PW_W3A + NCOP], start=True, stop=False
        )
        nc.tensor.matmul(
            psg[:], h2T[1][:], pw_t[:, PW_W3B : PW_W3B + NCOP], start=False, stop=False
        )
        # += ones[100,1] @ b3[1,112]: bias along the free dim via PE
        nc.tensor.matmul(
            psg[:], ones_bf[0:1, 0:NE], pw_t[0:1, PW_B3 : PW_B3 + NCOP],
            start=False, stop=True,
        )

        # ---- expand gates [100, 112] -> [100, 240] bf16 (irrep index map)
        # fused into PSUM->SBUF copies with broadcast source APs. Row 0 is
        # expanded first so chunk 0's partition_broadcast (and with it the
        # first store) doesn't wait on the full expansion.
        gexp = mlp_pool.tile([NE, D], BF16, tag="gexp")
        for r0, r1 in [(0, NE)]:
            nr = r1 - r0
            nc.scalar.copy(gexp[r0:r1, 0:64], psg[r0:r1, 0:64])
            nc.scalar.copy(
                gexp[r0:r1, 64:160].rearrange("e (i k) -> e i k", k=3),
                psg[r0:r1, 64:96].unsqueeze(2).to_broadcast((nr, 32, 3)),
            )
            nc.scalar.copy(
                gexp[r0:r1, 160:240].rearrange("e (i k) -> e i k", k=5),
                psg[r0:r1, 96:112].unsqueeze(2).to_broadcast((nr, 16, 5)),
            )

        # ---- broadcast expanded gates to all 128 partitions (see module
        # docstring): chunk 0 straight off partition 0, chunk 1 via
        # flatten + GPSIMD, chunks 2+ via PE+ACT pieces.
        raws = []
        for ci, (lo, hi) in enumerate(CHUNKS):
            raw = raw_pool.tile([128, (hi - lo) * D], BF16, tag=f"raw{ci}")
            raws.append(raw)

        nc.gpsimd.partition_broadcast(raws[0][:], gexp[0:1, :])

        for gi in range(1, NGP):
            logi, higi = CHUNKS[gi]
            stg = st_pool.tile(
                [1, (higi - logi) * D], BF16, name=f"st{gi}", tag=f"st{gi}"
            )
            nc.scalar.dma_start(out=stg[:], in_=gexp[logi:higi, :])
            nc.gpsimd.partition_broadcast(raws[gi][:], stg[0:1, :])

        fbase = CHUNKS[NGP][0]  # first PE+ACT e-row
        flat = st_pool.tile([1, (NE - fbase) * D], BF16)
        nc.sync.dma_start(out=flat[:], in_=gexp[fbase:NE, :])

        def bcast_pieces(ci):
            """PE+ACT broadcast of chunk ci (1024-col PSUM pieces)."""
            lo, hi = CHUNKS[ci]
            n = (hi - lo) * D
            base = (lo - fbase) * D
            for p0 in range(0, n, 1024):
                w = min(1024, n - p0)
                ps = psum_bc.tile(
                    [128, 1024], FP32, name=f"psbc{ci}_{p0}", tag="psbc"
                )
                for q0 in range(0, w, 512):
                    qw = min(512, w - q0)
                    nc.tensor.matmul(
                        ps[:, q0 : q0 + qw],
                        ones_bf[0:1, :],
                        flat[0:1, base + p0 + q0 : base + p0 + q0 + qw],
                        start=True, stop=True,
                    )
                nc.scalar.copy(raws[ci][:, p0 : p0 + w], ps[:, 0:w])

        # Emit ALL piece copies upfront, back-to-back: the ACT ring carries
        # no store DMAs (those all go on the sync ring), so the copies
        # stream 17..46 us and each chunk lands just before its multiplies
        # need it. (Interleaving copies between store DMA instructions
        # delayed chunk E's broadcast ~7 us and starved the store stream.)
        for ci in range(NGP, len(CHUNKS)):
            bcast_pieces(ci)

        # ---- main loop (chunk-major): out[m, e, d] = x[m, d] * gexp[e, d]
        # Chunk-major order so each chunk's multiplies start right after its
        # broadcast lands; stores alternate between both HWDGE rings. The
        # broadcast for chunk ci+2 is emitted between chunk ci's stores so
        # ACT-ring stores don't queue behind a full chunk of copies.
        si = 0
        for ci, (lo, hi) in enumerate(CHUNKS):
            ec = hi - lo
            n = ec * D
            for mt in range(MT):
                o_t = out_pool.tile([128, n], BF16)
                nc.vector.tensor_mul(
                    o_t[:].rearrange("p (e d) -> p e d", d=D),
                    x_t[mt][:].unsqueeze(1).to_broadcast((128, ec, D)),
                    raws[ci][:].rearrange("p (e d) -> p e d", d=D),
                )
                nc.sync.dma_start(
                    out=out_d[mt * 128 : (mt + 1) * 128, lo * D : hi * D],
                    in_=o_t[:],
                )
                si += 1

    nc.compile()
    return nc


def _marshal(inputs):
    f32 = lambda a: np.ascontiguousarray(np.asarray(a, dtype=np.float32))
    bf = lambda a: f32(a).astype(BF16_NP)
    x = bf(inputs["x"])
    W1, W2, W3 = bf(inputs["W1"]), bf(inputs["W2"]), bf(inputs["W3"])
    b1, b2, b3 = f32(inputs["b1"]), f32(inputs["b2"]), f32(inputs["b3"])
    eT = bf(np.asarray(inputs["e_feat"]).T)

    pw = np.zeros((128, NPW), BF16_NP)
    pw[:, PW_W2A : PW_W2A + 256] = W2[0:128]
    pw[:, PW_W2B : PW_W2B + 256] = W2[128:256]
    pw[:, PW_W3A : PW_W3A + NCOP] = W3[0:128]
    pw[:, PW_W3B : PW_W3B + NCOP] = W3[128:256]
    pw[0:64, PW_W1 : PW_W1 + 128] = W1[:, 0:128]
    pw[64:128, PW_W1 : PW_W1 + 128] = W1[:, 128:256]
    pw[0:64, PW_ET : PW_ET + NE] = eT
    pw[64:128, PW_ET : PW_ET + NE] = eT
    pw[0, PW_B3 : PW_B3 + NCOP] = b3.astype(BF16_NP)

    pf = np.zeros((128, NPF), np.float32)
    pf[:, 0] = b1[0:128]
    pf[:, 1] = b1[128:256]
    pf[:, 2] = b2[0:128]
    pf[:, 3] = b2[128:256]

    return [
        {"x": x[i * MC : (i + 1) * MC], "pweights": pw, "pbias": pf}
        for i in range(N_CORES)
    ]


def get_program():
    if "nc" not in _CACHE:
        _CACHE["nc"] = _build_program()
    return _CACHE["nc"]


def run(inputs, trace=False, **kwargs):
    """Run on 8 cores; returns (out [M, NE, D] f32, BassKernelResults)."""
    nc = get_program()
    in_maps = _marshal(inputs)
    res = run_bass_kernel_spmd(
        nc, in_maps, core_ids=list(range(N_CORES)), trace=trace, **kwargs
    )
    out = np.concatenate(
        [
            np.asarray(res.results[i]["out"])
            .astype(np.float32)
            .reshape(MC, NE, D)
            for i in range(N_CORES)
        ],
        axis=0,
    )
    return out, res


def kernel(**inputs) -> np.ndarray:
    out, _ = run(inputs)
    return out
